# revision 35
# baseline (speedup 1.0000x reference)
"""Trainium2 Bass kernel for a Bahdanau-attention 2-layer GRU decoder step.

Strategy (data-parallel over batch N across 8 NeuronCores, no collectives):
  - Host pre-arranges inputs: per-core encoder_states shard transposed to
    [d, (n, s)] layout (contraction dim on partitions), weights transposed +
    tile-blocked for the TensorEngine lhsT layout, bf16 for all big matmul
    operands, biases folded.
  - Attention: u = U_w @ enc (dominant matmul) accumulated per 512-wide free
    tile; w broadcast-added on DVE; tanh on ACT (bf16); energy via a bf16 PE
    matmul against attn_w; softmax over s on the free dim; context as a
    mul+reduce over s on DVE against a partition-broadcast attention copy.
  - GRU: the recurrent halves (W_hh @ h) are precomputed during attention,
    filling PE gaps while encoder chunks stream in; results park in SBUF and
    re-seed the gate PSUMs, so the serial GRU phase runs only the W_ih
    matmuls (weights prefetched through a deep SBUF ring during attention).
    Gate math fused on ACT/DVE, outputs transposed on PE per tile.
"""
import numpy as np
import ml_dtypes
from contextlib import ExitStack

import concourse.bass as bass
import concourse.tile as tile
from concourse import bacc, mybir
from concourse.bass_utils import run_bass_kernel_spmd
from concourse.masks import make_identity
from concourse.bass import ds, ts

F32 = mybir.dt.float32
BF16 = mybir.dt.bfloat16
AF = mybir.ActivationFunctionType
OP = mybir.AluOpType
AX = mybir.AxisListType

S, N, H, E, V = 64, 256, 1024, 256, 128
D2, DIN, H3 = 2048, 2304, 3072
NC = 8
NS = N // NC            # 32: per-core batch shard
SN = NS * S             # 2048: flattened (n, s), n-major
P = 128
KT_ENC = D2 // P        # 16
KT_H = H // P           # 8
KT_DIN = DIN // P       # 18
MT_H = H // P           # 8 h m-tiles
FT = 512                # moving free tile for the u matmul
NT = SN // FT           # 4
NB = FT // S            # 8 batch elements per free tile
NG = 3 * MT_H           # 24 gate m-tiles per layer


def build_program():
    nc = bacc.Bacc("TRN2", target_bir_lowering=False, debug=False,
                   num_devices=NC)

    def inp(name, shape, dtype=F32):
        return nc.dram_tensor(name, shape, dtype, kind="ExternalInput").ap()

    def outp(name, shape, dtype=F32):
        return nc.dram_tensor(name, shape, dtype, kind="ExternalOutput").ap()

    encT = inp("encT", [P, KT_ENC, SN], BF16)
    uwT = inp("uwT", [P, MT_H, KT_ENC, P], BF16)
    wwT = inp("wwT", [P, MT_H, KT_H, P], BF16)
    h0T_b = inp("h0T_b", [P, KT_H, NS], BF16)
    h1T_b = inp("h1T_b", [P, KT_H, NS], BF16)
    h0T_f = inp("h0T_f", [P, KT_H, NS])
    h1T_f = inp("h1T_f", [P, KT_H, NS])
    ubw = inp("ubw", [P, MT_H, 1])
    attw = inp("attw", [P, MT_H, 1], BF16)
    onehot = inp("onehot", [P, NS])
    emb = inp("emb", [P, 2, P])
    wih0T = inp("wih0T", [P, NG, KT_DIN, P], BF16)
    whh0T = inp("whh0T", [P, NG, KT_H, P], BF16)
    wih1T = inp("wih1T", [P, NG, KT_H, P], BF16)
    whh1T = inp("whh1T", [P, NG, KT_H, P], BF16)
    brz0 = inp("brz0", [P, 2 * MT_H, 1])
    bin0 = inp("bin0", [P, MT_H, 1])
    bhn0 = inp("bhn0", [P, MT_H, 1])
    brz1 = inp("brz1", [P, 2 * MT_H, 1])
    bin1 = inp("bin1", [P, MT_H, 1])
    bhn1 = inp("bhn1", [P, MT_H, 1])
    fcwT = inp("fcwT", [P, KT_H, P])
    fcb = inp("fcb", [P, 1])

    pred_out = outp("pred_out", [NS, V])
    hid_out = outp("hid_out", [2, NS, H])
    att_out = outp("att_out", [S, NS])

    with tile.TileContext(nc) as tc, ExitStack() as ctx:
        const = ctx.enter_context(tc.tile_pool(name="const", bufs=1))
        psums = ctx.enter_context(tc.tile_pool(name="psums", bufs=3, space="PSUM"))
        gwork = ctx.enter_context(tc.tile_pool(name="gwork", bufs=4))
        gw = ctx.enter_context(tc.tile_pool(name="gw", bufs=12))
        gwhh = ctx.enter_context(tc.tile_pool(name="gwhh", bufs=5))

        def cload(ap_in, shape, dtype=F32):
            t = const.tile(shape, dtype, tag=ap_in.tensor.name)
            nc.sync.dma_start(t[:], ap_in[:])
            return t

        h0b_sb = cload(h0T_b, [P, KT_H, NS], BF16)
        h1b_sb = cload(h1T_b, [P, KT_H, NS], BF16)
        h0f_sb = cload(h0T_f, [P, KT_H, NS])
        h1f_sb = cload(h1T_f, [P, KT_H, NS])
        ubw_sb = cload(ubw, [P, MT_H, 1])
        attw_sb = cload(attw, [P, MT_H, 1], BF16)
        onehot_sb = cload(onehot, [P, NS])
        emb_sb = cload(emb, [P, 2, P])
        brz0_sb = cload(brz0, [P, 2 * MT_H, 1])
        bin0_sb = cload(bin0, [P, MT_H, 1])
        bhn0_sb = cload(bhn0, [P, MT_H, 1])
        brz1_sb = cload(brz1, [P, 2 * MT_H, 1])
        bin1_sb = cload(bin1, [P, MT_H, 1])
        bhn1_sb = cload(bhn1, [P, MT_H, 1])
        fcw_sb = cload(fcwT, [P, KT_H, P])
        fcb_sb = cload(fcb, [P, 1])
        ident = const.tile([P, P], F32)
        make_identity(nc, ident)

        rnn_bf = const.tile([P, KT_DIN, NS], BF16)
        att_tr = const.tile([S, NS], F32)
        # precomputed recurrent halves: L0 r/z/n then L1 r/z/n
        ghsb = const.tile([P, 2 * NG, NS], F32)

        def gh_group(whh_src, g, out_idx, hb):
            c = gwhh.tile([P, KT_H, P], BF16, tag="whh")
            nc.sync.dma_start(c[:], whh_src[:, g, :, :])
            pg = psums.tile([P, NS], F32, tag="small")
            for kt in range(KT_H):
                nc.tensor.matmul(pg[:], c[:, kt, :], hb[:, kt, :],
                                 start=(kt == 0), stop=(kt == KT_H - 1))
            nc.scalar.activation(ghsb[:, out_idx, :], pg[:], AF.Identity)

        gh_jobs = ([(whh0T, g, g, h0b_sb) for g in range(NG)] +
                   [(whh1T, g, NG + g, h1b_sb) for g in range(NG)])

        # ================= attention phase (scoped SBUF) =================
        with tc.tile_pool(name="aenc", bufs=2) as aenc, \
             tc.tile_pool(name="auw", bufs=1) as auw, \
             tc.tile_pool(name="ares", bufs=1) as ares, \
             tc.tile_pool(name="apsum", bufs=5, space="PSUM") as apsum, \
             tc.tile_pool(name="awork", bufs=2) as awork, \
             tc.tile_pool(name="aprod", bufs=1) as aprod, \
             tc.tile_pool(name="attp", bufs=2) as attp, \
             tc.tile_pool(name="adram", bufs=2, space="DRAM") as adram:

            # per-(kt, t) enc chunks and per-m U_w tiles: fine-grained DMA
            # deps so the first u matmuls start as soon as chunks land.
            uw_sb = [None] * MT_H
            uw_sb[0] = auw.tile([P, KT_ENC, P], BF16, tag="uw0", name="uw0")
            nc.sync.dma_start(uw_sb[0][:], uwT[:, 0, :, :])
            # enc chunks are double-buffered just-in-time per (kt, t): tile
            # (kt, t+1) loads while t computes, and (kt, t) frees after the
            # context pass reads it.
            enc_sb = [[None] * NT for _ in range(KT_ENC)]

            def load_enc_chunks(t):
                for kt in range(KT_ENC):
                    tt = aenc.tile([P, FT], BF16, tag=f"enc{kt}",
                                   name=f"enc{kt}_{t}")
                    nc.sync.dma_start(tt[:], encT[:, kt, ds(t * FT, FT)])
                    enc_sb[kt][t] = tt

            load_enc_chunks(0)
            for m in range(1, MT_H):
                uw_sb[m] = auw.tile([P, KT_ENC, P], BF16,
                                    tag=f"uw{m}", name=f"uw{m}")
                nc.sync.dma_start(uw_sb[m][:], uwT[:, m, :, :])

            # w_T[h, n] = W_w @ h1_shard + (U_b + W_b)
            w_sb = ares.tile([P, MT_H, NS], F32)
            for m in range(MT_H):
                ww_c = awork.tile([P, KT_H, P], BF16, tag="wwc")
                nc.sync.dma_start(ww_c[:], wwT[:, m, :, :])
                pw = psums.tile([P, NS], F32, tag="small")
                for kt in range(KT_H):
                    nc.tensor.matmul(pw[:], ww_c[:, kt, :], h1b_sb[:, kt, :],
                                     start=(kt == 0), stop=(kt == KT_H - 1))
                nc.scalar.activation(w_sb[:, m, :], pw[:], AF.Identity,
                                     bias=ubw_sb[:, m, :])

            # embedding gather via one-hot matmul (exact in fp32)
            for et in range(2):
                pe_ = psums.tile([P, NS], F32, tag="small")
                nc.tensor.matmul(pe_[:], emb_sb[:, et, :], onehot_sb[:],
                                 start=True, stop=True)
                nc.vector.tensor_copy(rnn_bf[:, KT_ENC + et, :], pe_[:])

            # first slice of recurrent-half precompute (fills PE startup)
            for job in gh_jobs[0:12]:
                gh_group(*job)

            # queue the W_ih stream (ring prefetches during attention)
            def wih_chunk(src, g, kt_in):
                t = gw.tile([P, kt_in, P], BF16, tag="wih")
                nc.sync.dma_start(t[:], src[:, g, :, :])
                return t

            l0_chunks = [(wih_chunk(wih0T, j, KT_DIN),
                          wih_chunk(wih0T, MT_H + j, KT_DIN),
                          wih_chunk(wih0T, 2 * MT_H + j, KT_DIN))
                         for j in range(MT_H)]
            l1_chunks = [(wih_chunk(wih1T, j, KT_H),
                          wih_chunk(wih1T, MT_H + j, KT_H),
                          wih_chunk(wih1T, 2 * MT_H + j, KT_H))
                         for j in range(MT_H)]

            for t in range(NT):
                if t + 1 < NT:
                    load_enc_chunks(t + 1)
                pe_en = psums.tile([1, FT], F32, tag="small")
                for m in range(MT_H):
                    pu = apsum.tile([P, FT], F32, tag="pu")
                    for kt in range(KT_ENC):
                        nc.tensor.matmul(pu[:], uw_sb[m][:, kt, :],
                                         enc_sb[kt][t][:],
                                         start=(kt == 0),
                                         stop=(kt == KT_ENC - 1))
                    tmp = awork.tile([P, NB, S], BF16, tag="tmp")
                    nc.vector.tensor_tensor(
                        tmp[:], pu[:].rearrange("p (n s) -> p n s", s=S),
                        w_sb[:, m, ds(t * NB, NB), None].to_broadcast(
                            [P, NB, S]),
                        OP.add)
                    th = awork.tile([P, FT], BF16, tag="th")
                    nc.scalar.activation(
                        th[:], tmp[:].rearrange("p n s -> p (n s)"), AF.Tanh)
                    nc.tensor.matmul(pe_en[:], attw_sb[:, m, :], th[:],
                                     start=(m == 0), stop=(m == MT_H - 1),
                                     skip_group_check=True)

                # energy [1, 512] -> [8, 64] (n-part, s-free), softmax over s
                en_sb = attp.tile([1, FT], F32, tag="en")
                nc.vector.tensor_copy(en_sb[:], pe_en[:])
                en_nm = attp.tile([NB, S], F32, tag="ennm")
                nc.sync.dma_start(en_nm[:], en_sb[:])
                mxn = attp.tile([NB, 1], F32, tag="mxn")
                nc.vector.reduce_max(out=mxn[:], in_=en_nm[:], axis=AX.X,
                                     negate=True)
                ex = attp.tile([NB, S], F32, tag="ex")
                se = attp.tile([NB, 1], F32, tag="se")
                nc.scalar.activation(ex[:], en_nm[:], AF.Exp, bias=mxn[:],
                                     accum_out=se[:])
                rc = attp.tile([NB, 1], F32, tag="rc")
                nc.vector.reciprocal(rc[:], se[:])
                att_t = attp.tile([NB, S], F32, tag="attt")
                nc.vector.tensor_scalar_mul(att_t[:], ex[:], rc[:])
                pat = psums.tile([S, NB], F32, tag="small")
                nc.tensor.transpose(pat[:], att_t[:], ident[:NB, :NB])
                nc.vector.tensor_copy(att_tr[:, ds(t * NB, NB)], pat[:])

                # broadcast att across partitions via DRAM bounce (bf16)
                att_tb = attp.tile([NB, S], BF16, tag="attb16")
                with nc.allow_low_precision(reason="attention weights bf16"):
                    nc.vector.tensor_copy(att_tb[:], att_t[:])
                att_d = adram.tile([FT], BF16, tag="attd")
                nc.sync.dma_start(att_d[:], att_tb[:])
                att_b = awork.tile([P, FT], BF16, tag="attb")
                nc.sync.dma_start(att_b[:],
                                  att_d[None, :].to_broadcast([P, FT]))

                # context chunk: rnn[d, n(t)] = sum_s enc[d, n, s] * att[n, s]
                # (runs on the otherwise-idle GpSimd so the DVE queue never
                # blocks PSUM recycling for the u matmuls)
                att_bv = att_b[:].rearrange("p (n s) -> p n s", s=S)
                KH = KT_ENC // 2
                for h2 in range(2):
                    prod = aprod.tile([P, KH, NB, S], BF16, tag="prod")
                    for kk in range(KH):
                        kt = h2 * KH + kk
                        nc.gpsimd.tensor_tensor(
                            prod[:, kk],
                            enc_sb[kt][t][:].rearrange("p (n s) -> p n s",
                                                       s=S),
                            att_bv, OP.mult)
                    with nc.allow_low_precision(
                            reason="fp32 accumulate, bf16 final write"):
                        nc.vector.reduce_sum(
                            out=rnn_bf[:, ds(h2 * KH, KH), ds(t * NB, NB)],
                            in_=prod[:], axis=AX.X)

                # next slice of recurrent-half precompute (fills PE gaps)
                if t < NT - 1:
                    for job in gh_jobs[12 * (t + 1):12 * (t + 2)]:
                        gh_group(*job)

        # ================= GRU phase (gi matmuls only) ====================
        def gru_layer(chunks, kt_in, rhs_bf, hf_sb, ghbase,
                      brz_sb, bin_sb, bhn_sb, out_f, out_bf, hid_l):
            for j in range(MT_H):
                wr, wz, wn = chunks[j]

                def gates(wi, ghidx, bias_ap, func, tag):
                    pg = psums.tile([P, NS], F32, tag="small")
                    for kt in range(kt_in):
                        nc.tensor.matmul(pg[:], wi[:, kt, :], rhs_bf[:, kt, :],
                                         start=(kt == 0), stop=(kt == kt_in - 1))
                    gs = gwork.tile([P, NS], F32, tag=tag + "s")
                    nc.vector.tensor_add(gs[:], pg[:], ghsb[:, ghidx, :])
                    g = gwork.tile([P, NS], F32, tag=tag)
                    nc.scalar.activation(g[:], gs[:], func, bias=bias_ap)
                    return g

                r_t = gates(wr, ghbase + j, brz_sb[:, j, :], AF.Sigmoid, "r")
                z_t = gates(wz, ghbase + MT_H + j, brz_sb[:, MT_H + j, :],
                            AF.Sigmoid, "z")

                pin = psums.tile([P, NS], F32, tag="small")
                for kt in range(kt_in):
                    nc.tensor.matmul(pin[:], wn[:, kt, :], rhs_bf[:, kt, :],
                                     start=(kt == 0), stop=(kt == kt_in - 1))
                hn_t = gwork.tile([P, NS], F32, tag="hn")
                nc.scalar.activation(hn_t[:], ghsb[:, ghbase + 2 * MT_H + j, :],
                                     AF.Identity, bias=bhn_sb[:, j, :])
                t1 = gwork.tile([P, NS], F32, tag="t1")
                nc.vector.tensor_mul(t1[:], r_t[:], hn_t[:])
                t2 = gwork.tile([P, NS], F32, tag="t2")
                nc.vector.tensor_add(t2[:], pin[:], t1[:])
                n_t = gwork.tile([P, NS], F32, tag="nt")
                nc.scalar.activation(n_t[:], t2[:], AF.Tanh,
                                     bias=bin_sb[:, j, :])
                d_t = gwork.tile([P, NS], F32, tag="dt")
                nc.vector.tensor_sub(d_t[:], hf_sb[:, j, :], n_t[:])
                zd = gwork.tile([P, NS], F32, tag="zd")
                nc.vector.tensor_mul(zd[:], z_t[:], d_t[:])
                nc.vector.tensor_add(out_f[:, j, :], n_t[:], zd[:])
                if out_bf is not None:
                    with nc.allow_low_precision(reason="bf16 matmul rhs copy"):
                        nc.vector.tensor_copy(out_bf[:, j, :], out_f[:, j, :])
                # transpose + write this hidden tile now (overlaps next j)
                pt = psums.tile([NS, P], F32, tag="small")
                nc.tensor.transpose(pt[:], out_f[:, j, :], ident[:])
                hid_nat = gwork.tile([NS, P], F32, tag="hidn")
                nc.vector.tensor_copy(hid_nat[:], pt[:])
                nc.sync.dma_start(hid_out[hid_l, :, ds(j * P, P)], hid_nat[:])

        with tc.tile_pool(name="gres", bufs=1) as gres:
            h0p_f = gres.tile([P, KT_H, NS], F32)
            h0p_bf = gres.tile([P, KT_H, NS], BF16)
            h1p_f = gres.tile([P, KT_H, NS], F32)
            gru_layer(l0_chunks, KT_DIN, rnn_bf, h0f_sb, 0,
                      brz0_sb, bin0_sb, bhn0_sb, h0p_f, h0p_bf, 0)
            gru_layer(l1_chunks, KT_H, h0p_bf, h1f_sb, NG,
                      brz1_sb, bin1_sb, bhn1_sb, h1p_f, None, 1)

            # ---- fc (fp32) ----
            pfc = psums.tile([P, NS], F32, tag="small")
            for kt in range(KT_H):
                nc.tensor.matmul(pfc[:], fcw_sb[:, kt, :], h1p_f[:, kt, :],
                                 start=(kt == 0), stop=(kt == KT_H - 1))
            pred_sb = gwork.tile([P, NS], F32, tag="pred")
            nc.scalar.activation(pred_sb[:], pfc[:], AF.Identity,
                                 bias=fcb_sb[:])
            ptr = psums.tile([NS, P], F32, tag="small")
            nc.tensor.transpose(ptr[:], pred_sb[:], ident[:])
            pred_t = gwork.tile([NS, P], F32, tag="predt")
            nc.vector.tensor_copy(pred_t[:], ptr[:])
            nc.sync.dma_start(pred_out[:], pred_t[:])

            nc.sync.dma_start(att_out[:], att_tr[:])

    nc.compile()
    return nc


def _blocked_lhsT(w, n_mt, n_kt, dtype):
    # w: (n_mt*128, n_kt*128) row-major -> [P(k), n_mt, n_kt, P(m)]
    b = w.reshape(n_mt, P, n_kt, P).transpose(3, 0, 2, 1)
    return np.ascontiguousarray(b).astype(dtype)


def _kstack(t, n_kt, dtype):
    # t: (n_kt*128, F) -> [P, n_kt, F]
    b = t.reshape(n_kt, P, -1).transpose(1, 0, 2)
    return np.ascontiguousarray(b).astype(dtype)


def _colvec(v, n_t):
    return np.ascontiguousarray(
        np.asarray(v, np.float32).reshape(n_t, P).T[:, :, None])


BF = ml_dtypes.bfloat16
_prog_cache = {}


def _get_program():
    if "nc" not in _prog_cache:
        _prog_cache["nc"] = build_program()
    return _prog_cache["nc"]


def make_in_maps(x, encoder_states, hidden, emb, U_w, U_b, W_w, W_b,
                 attn_w, W_ih0, W_hh0, b_ih0, b_hh0,
                 W_ih1, W_hh1, b_ih1, b_hh1, fc_w, fc_b):
    f = np.float32
    x = np.asarray(x)
    enc = np.asarray(encoder_states, f)
    hidden = np.asarray(hidden, f)
    shared = {
        "uwT": _blocked_lhsT(np.asarray(U_w, f), MT_H, KT_ENC, BF),
        "wwT": _blocked_lhsT(np.asarray(W_w, f), MT_H, KT_H, BF),
        "ubw": _colvec(np.asarray(U_b, f) + np.asarray(W_b, f), MT_H),
        "attw": _colvec(np.asarray(attn_w, f)[0], MT_H).astype(BF),
        "emb": np.ascontiguousarray(np.asarray(emb, f).reshape(P, 2, P)),
        "wih0T": _blocked_lhsT(np.asarray(W_ih0, f), NG, KT_DIN, BF),
        "whh0T": _blocked_lhsT(np.asarray(W_hh0, f), NG, KT_H, BF),
        "wih1T": _blocked_lhsT(np.asarray(W_ih1, f), NG, KT_H, BF),
        "whh1T": _blocked_lhsT(np.asarray(W_hh1, f), NG, KT_H, BF),
        "brz0": _colvec((np.asarray(b_ih0, f) + np.asarray(b_hh0, f))[:2 * H],
                        2 * MT_H),
        "bin0": _colvec(np.asarray(b_ih0, f)[2 * H:], MT_H),
        "bhn0": _colvec(np.asarray(b_hh0, f)[2 * H:], MT_H),
        "brz1": _colvec((np.asarray(b_ih1, f) + np.asarray(b_hh1, f))[:2 * H],
                        2 * MT_H),
        "bin1": _colvec(np.asarray(b_ih1, f)[2 * H:], MT_H),
        "bhn1": _colvec(np.asarray(b_hh1, f)[2 * H:], MT_H),
        "fcwT": _blocked_lhsT(np.asarray(fc_w, f), 1, KT_H, f)[:, 0],
        "fcb": np.ascontiguousarray(np.asarray(fc_b, f)[:, None]),
    }
    in_maps = []
    for k in range(NC):
        n0 = k * NS
        encs = enc[:, n0:n0 + NS, :]                      # (S, NS, 2H)
        encT2 = encs.transpose(2, 1, 0).reshape(D2, SN)   # [d, (n s)]
        h0 = hidden[0, n0:n0 + NS].T                      # (H, NS)
        h1 = hidden[1, n0:n0 + NS].T
        xs = np.asarray(x[n0:n0 + NS])
        onehot = (xs[None, :] == np.arange(V)[:, None]).astype(f)
        m = dict(shared)
        m.update({
            "encT": _kstack(encT2, KT_ENC, BF),
            "h0T_b": _kstack(h0, KT_H, BF),
            "h1T_b": _kstack(h1, KT_H, BF),
            "h0T_f": _kstack(h0, KT_H, f),
            "h1T_f": _kstack(h1, KT_H, f),
            "onehot": np.ascontiguousarray(onehot),
        })
        in_maps.append(m)
    return in_maps


def assemble_outputs(results):
    pred = np.concatenate([r["pred_out"] for r in results], axis=0)
    hid = np.concatenate([r["hid_out"] for r in results], axis=1)
    att = np.concatenate([r["att_out"] for r in results], axis=1)[:, :, None]
    return (pred.astype(np.float32), hid.astype(np.float32),
            att.astype(np.float32))


def kernel(x, encoder_states, hidden, cell, emb, U_w, U_b, W_w, W_b,
           attn_w, attn_b, W_ih0, W_hh0, b_ih0, b_hh0,
           W_ih1, W_hh1, b_ih1, b_hh1, fc_w, fc_b, **_ignored):
    # attn_b shifts every energy equally -> softmax-invariant; cell unused.
    nc = _get_program()
    in_maps = make_in_maps(x, encoder_states, hidden, emb, U_w, U_b, W_w, W_b,
                           attn_w, W_ih0, W_hh0, b_ih0, b_hh0,
                           W_ih1, W_hh1, b_ih1, b_hh1, fc_w, fc_b)
    res = run_bass_kernel_spmd(nc, in_maps, core_ids=list(range(NC)))
    return assemble_outputs(res.results)


# revision 36
# speedup vs baseline: 1.1253x; 1.1253x over previous
"""Trainium2 Bass kernel for a Bahdanau-attention 2-layer GRU decoder step.

Strategy (data-parallel over batch N across 8 NeuronCores, no collectives):
  - Host pre-arranges inputs: per-core encoder_states shard transposed to
    [d, (n, s)] layout (contraction dim on partitions), weights transposed +
    tile-blocked for the TensorEngine lhsT layout, bf16 for all big matmul
    operands, biases folded.
  - Attention: u = U_w @ enc (dominant matmul) accumulated per 512-wide free
    tile; w broadcast-added on DVE; tanh on ACT (bf16); energy via a bf16 PE
    matmul against attn_w; softmax over s on the free dim; context as a
    mul+reduce over s on DVE against a partition-broadcast attention copy.
  - GRU: the recurrent halves (W_hh @ h) are precomputed during attention,
    filling PE gaps while encoder chunks stream in; results park in SBUF and
    re-seed the gate PSUMs, so the serial GRU phase runs only the W_ih
    matmuls (weights prefetched through a deep SBUF ring during attention).
    Gate math fused on ACT/DVE, outputs transposed on PE per tile.
"""
import numpy as np
import ml_dtypes
from contextlib import ExitStack

import concourse.bass as bass
import concourse.tile as tile
from concourse import bacc, mybir
from concourse.bass_utils import run_bass_kernel_spmd
from concourse.masks import make_identity
from concourse.bass import ds, ts

F32 = mybir.dt.float32
BF16 = mybir.dt.bfloat16
AF = mybir.ActivationFunctionType
OP = mybir.AluOpType
AX = mybir.AxisListType

S, N, H, E, V = 64, 256, 1024, 256, 128
D2, DIN, H3 = 2048, 2304, 3072
NC = 8
NS = N // NC            # 32: per-core batch shard
SN = NS * S             # 2048: flattened (n, s), n-major
P = 128
KT_ENC = D2 // P        # 16
KT_H = H // P           # 8
KT_DIN = DIN // P       # 18
MT_H = H // P           # 8 h m-tiles
FT = 512                # moving free tile for the u matmul
NT = SN // FT           # 4
NB = FT // S            # 8 batch elements per free tile
NG = 3 * MT_H           # 24 gate m-tiles per layer


def build_program():
    nc = bacc.Bacc("TRN2", target_bir_lowering=False, debug=False,
                   num_devices=NC)

    def inp(name, shape, dtype=F32):
        return nc.dram_tensor(name, shape, dtype, kind="ExternalInput").ap()

    def outp(name, shape, dtype=F32):
        return nc.dram_tensor(name, shape, dtype, kind="ExternalOutput").ap()

    encT = inp("encT", [P, KT_ENC, SN], BF16)
    uwT = inp("uwT", [P, MT_H, KT_ENC, P], BF16)
    wwT = inp("wwT", [P, MT_H, KT_H, P], BF16)
    h0T_b = inp("h0T_b", [P, KT_H, NS], BF16)
    h1T_b = inp("h1T_b", [P, KT_H, NS], BF16)
    h0T_f = inp("h0T_f", [P, KT_H, NS])
    h1T_f = inp("h1T_f", [P, KT_H, NS])
    ubw = inp("ubw", [P, MT_H, 1])
    attw = inp("attw", [P, MT_H, 1], BF16)
    onehot = inp("onehot", [P, NS])
    emb = inp("emb", [P, 2, P])
    wih0T = inp("wih0T", [P, NG, KT_DIN, P], BF16)
    whh0T = inp("whh0T", [P, NG, KT_H, P], BF16)
    wih1T = inp("wih1T", [P, NG, KT_H, P], BF16)
    whh1T = inp("whh1T", [P, NG, KT_H, P], BF16)
    brz0 = inp("brz0", [P, 2 * MT_H, 1])
    bin0 = inp("bin0", [P, MT_H, 1])
    bhn0 = inp("bhn0", [P, MT_H, 1])
    brz1 = inp("brz1", [P, 2 * MT_H, 1])
    bin1 = inp("bin1", [P, MT_H, 1])
    bhn1 = inp("bhn1", [P, MT_H, 1])
    fcwT = inp("fcwT", [P, KT_H, P])
    fcb = inp("fcb", [P, 1])

    pred_out = outp("pred_out", [NS, V])
    hid_out = outp("hid_out", [2, NS, H])
    att_out = outp("att_out", [S, NS])

    with tile.TileContext(nc) as tc, ExitStack() as ctx:
        const = ctx.enter_context(tc.tile_pool(name="const", bufs=1))
        psums = ctx.enter_context(tc.tile_pool(name="psums", bufs=3, space="PSUM"))
        gwork = ctx.enter_context(tc.tile_pool(name="gwork", bufs=4))
        gw = ctx.enter_context(tc.tile_pool(name="gw", bufs=12))
        gwhh = ctx.enter_context(tc.tile_pool(name="gwhh", bufs=5))

        def cload(ap_in, shape, dtype=F32):
            t = const.tile(shape, dtype, tag=ap_in.tensor.name)
            nc.sync.dma_start(t[:], ap_in[:])
            return t

        h0b_sb = cload(h0T_b, [P, KT_H, NS], BF16)
        h1b_sb = cload(h1T_b, [P, KT_H, NS], BF16)
        h0f_sb = cload(h0T_f, [P, KT_H, NS])
        h1f_sb = cload(h1T_f, [P, KT_H, NS])
        ubw_sb = cload(ubw, [P, MT_H, 1])
        attw_sb = cload(attw, [P, MT_H, 1], BF16)
        onehot_sb = cload(onehot, [P, NS])
        emb_sb = cload(emb, [P, 2, P])
        brz0_sb = cload(brz0, [P, 2 * MT_H, 1])
        bin0_sb = cload(bin0, [P, MT_H, 1])
        bhn0_sb = cload(bhn0, [P, MT_H, 1])
        brz1_sb = cload(brz1, [P, 2 * MT_H, 1])
        bin1_sb = cload(bin1, [P, MT_H, 1])
        bhn1_sb = cload(bhn1, [P, MT_H, 1])
        fcw_sb = cload(fcwT, [P, KT_H, P])
        fcb_sb = cload(fcb, [P, 1])
        ident = const.tile([P, P], F32)
        make_identity(nc, ident)

        rnn_bf = const.tile([P, KT_DIN, NS], BF16)
        att_tr = const.tile([S, NS], F32)
        # precomputed recurrent halves: L0 r/z/n then L1 r/z/n
        ghsb = const.tile([P, 2 * NG, NS], F32)

        def gh_group(whh_src, g, out_idx, hb):
            c = gwhh.tile([P, KT_H, P], BF16, tag="whh")
            nc.sync.dma_start(c[:], whh_src[:, g, :, :])
            pg = psums.tile([P, NS], F32, tag="small")
            for kt in range(KT_H):
                nc.tensor.matmul(pg[:], c[:, kt, :], hb[:, kt, :],
                                 start=(kt == 0), stop=(kt == KT_H - 1))
            nc.scalar.activation(ghsb[:, out_idx, :], pg[:], AF.Identity)

        gh_jobs = ([(whh0T, g, g, h0b_sb) for g in range(NG)] +
                   [(whh1T, g, NG + g, h1b_sb) for g in range(NG)])

        # ================= attention phase (scoped SBUF) =================
        with tc.tile_pool(name="aenc", bufs=2) as aenc, \
             tc.tile_pool(name="auw", bufs=1) as auw, \
             tc.tile_pool(name="ares", bufs=1) as ares, \
             tc.tile_pool(name="apsum", bufs=5, space="PSUM") as apsum, \
             tc.tile_pool(name="awork", bufs=2) as awork, \
             tc.tile_pool(name="aprod", bufs=1) as aprod, \
             tc.tile_pool(name="attp", bufs=2) as attp, \
             tc.tile_pool(name="adram", bufs=2, space="DRAM") as adram:

            # per-(kt, t) enc chunks and per-m U_w tiles: fine-grained DMA
            # deps so the first u matmuls start as soon as chunks land.
            uw_sb = [None] * MT_H
            uw_sb[0] = auw.tile([P, KT_ENC, P], BF16, tag="uw0", name="uw0")
            nc.sync.dma_start(uw_sb[0][:], uwT[:, 0, :, :])
            # enc chunks are double-buffered just-in-time per (kt, t): tile
            # (kt, t+1) loads while t computes, and (kt, t) frees after the
            # context pass reads it.
            enc_sb = [[None] * NT for _ in range(KT_ENC)]

            def load_enc_chunks(t):
                for kt in range(KT_ENC):
                    tt = aenc.tile([P, FT], BF16, tag=f"enc{kt}",
                                   name=f"enc{kt}_{t}")
                    nc.sync.dma_start(tt[:], encT[:, kt, ds(t * FT, FT)])
                    enc_sb[kt][t] = tt

            load_enc_chunks(0)
            for m in range(1, MT_H):
                uw_sb[m] = auw.tile([P, KT_ENC, P], BF16,
                                    tag=f"uw{m}", name=f"uw{m}")
                nc.sync.dma_start(uw_sb[m][:], uwT[:, m, :, :])

            # w_T[h, n] = W_w @ h1_shard + (U_b + W_b)
            w_sb = ares.tile([P, MT_H, NS], F32)
            for m in range(MT_H):
                ww_c = awork.tile([P, KT_H, P], BF16, tag="wwc")
                nc.sync.dma_start(ww_c[:], wwT[:, m, :, :])
                pw = psums.tile([P, NS], F32, tag="small")
                for kt in range(KT_H):
                    nc.tensor.matmul(pw[:], ww_c[:, kt, :], h1b_sb[:, kt, :],
                                     start=(kt == 0), stop=(kt == KT_H - 1))
                nc.scalar.activation(w_sb[:, m, :], pw[:], AF.Identity,
                                     bias=ubw_sb[:, m, :])

            # embedding gather via one-hot matmul (exact in fp32)
            for et in range(2):
                pe_ = psums.tile([P, NS], F32, tag="small")
                nc.tensor.matmul(pe_[:], emb_sb[:, et, :], onehot_sb[:],
                                 start=True, stop=True)
                nc.vector.tensor_copy(rnn_bf[:, KT_ENC + et, :], pe_[:])

            # first slice of recurrent-half precompute (fills PE startup)
            for job in gh_jobs[0:12]:
                gh_group(*job)

            # queue the W_ih stream (ring prefetches during attention)
            def wih_chunk(src, g, kt_in):
                t = gw.tile([P, kt_in, P], BF16, tag="wih")
                nc.sync.dma_start(t[:], src[:, g, :, :])
                return t

            l0_chunks = [(wih_chunk(wih0T, j, KT_DIN),
                          wih_chunk(wih0T, MT_H + j, KT_DIN),
                          wih_chunk(wih0T, 2 * MT_H + j, KT_DIN))
                         for j in range(MT_H)]
            l1_chunks = [(wih_chunk(wih1T, j, KT_H),
                          wih_chunk(wih1T, MT_H + j, KT_H),
                          wih_chunk(wih1T, 2 * MT_H + j, KT_H))
                         for j in range(MT_H)]

            for t in range(NT):
                if t + 1 < NT:
                    load_enc_chunks(t + 1)
                pe_en = psums.tile([1, FT], F32, tag="small")
                for m in range(MT_H):
                    pu = apsum.tile([P, FT], F32, tag="pu")
                    for kt in range(KT_ENC):
                        nc.tensor.matmul(pu[:], uw_sb[m][:, kt, :],
                                         enc_sb[kt][t][:],
                                         start=(kt == 0),
                                         stop=(kt == KT_ENC - 1))
                    tmp = awork.tile([P, NB, S], BF16, tag="tmp")
                    nc.vector.tensor_tensor(
                        tmp[:], pu[:].rearrange("p (n s) -> p n s", s=S),
                        w_sb[:, m, ds(t * NB, NB), None].to_broadcast(
                            [P, NB, S]),
                        OP.add)
                    th = awork.tile([P, FT], BF16, tag="th")
                    nc.scalar.activation(
                        th[:], tmp[:].rearrange("p n s -> p (n s)"), AF.Tanh)
                    nc.tensor.matmul(pe_en[:], attw_sb[:, m, :], th[:],
                                     start=(m == 0), stop=(m == MT_H - 1),
                                     skip_group_check=True)

                # energy [1, 512] -> [8, 64] (n-part, s-free), softmax over s
                en_sb = attp.tile([1, FT], F32, tag="en")
                nc.vector.tensor_copy(en_sb[:], pe_en[:])
                en_nm = attp.tile([NB, S], F32, tag="ennm")
                nc.sync.dma_start(en_nm[:], en_sb[:])
                mxn = attp.tile([NB, 1], F32, tag="mxn")
                nc.vector.reduce_max(out=mxn[:], in_=en_nm[:], axis=AX.X,
                                     negate=True)
                ex = attp.tile([NB, S], F32, tag="ex")
                se = attp.tile([NB, 1], F32, tag="se")
                nc.scalar.activation(ex[:], en_nm[:], AF.Exp, bias=mxn[:],
                                     accum_out=se[:])
                rc = attp.tile([NB, 1], F32, tag="rc")
                nc.vector.reciprocal(rc[:], se[:])
                att_t = attp.tile([NB, S], F32, tag="attt")
                nc.vector.tensor_scalar_mul(att_t[:], ex[:], rc[:])
                pat = psums.tile([S, NB], F32, tag="small")
                nc.tensor.transpose(pat[:], att_t[:], ident[:NB, :NB])
                nc.vector.tensor_copy(att_tr[:, ds(t * NB, NB)], pat[:])

                # broadcast att across partitions via DRAM bounce (bf16)
                att_tb = attp.tile([NB, S], BF16, tag="attb16")
                with nc.allow_low_precision(reason="attention weights bf16"):
                    nc.vector.tensor_copy(att_tb[:], att_t[:])
                att_d = adram.tile([FT], BF16, tag="attd")
                nc.sync.dma_start(att_d[:], att_tb[:])
                att_b = awork.tile([P, FT], BF16, tag="attb")
                nc.sync.dma_start(att_b[:],
                                  att_d[None, :].to_broadcast([P, FT]))

                # context chunk: rnn[d, n(t)] = sum_s enc[d, n, s] * att[n, s]
                # (runs on the otherwise-idle GpSimd so the DVE queue never
                # blocks PSUM recycling for the u matmuls)
                att_bv = att_b[:].rearrange("p (n s) -> p n s", s=S)
                KH = KT_ENC // 2
                for h2 in range(2):
                    prod = aprod.tile([P, KH, NB, S], BF16, tag="prod")
                    for kk in range(KH):
                        kt = h2 * KH + kk
                        nc.vector.tensor_tensor(
                            prod[:, kk],
                            enc_sb[kt][t][:].rearrange("p (n s) -> p n s",
                                                       s=S),
                            att_bv, OP.mult)
                    with nc.allow_low_precision(
                            reason="fp32 accumulate, bf16 final write"):
                        nc.vector.reduce_sum(
                            out=rnn_bf[:, ds(h2 * KH, KH), ds(t * NB, NB)],
                            in_=prod[:], axis=AX.X)

                # next slice of recurrent-half precompute (fills PE gaps)
                if t < NT - 1:
                    for job in gh_jobs[12 * (t + 1):12 * (t + 2)]:
                        gh_group(*job)

        # ================= GRU phase (gi matmuls only) ====================
        def gru_layer(chunks, kt_in, rhs_bf, hf_sb, ghbase,
                      brz_sb, bin_sb, bhn_sb, out_f, out_bf, hid_l):
            for j in range(MT_H):
                wr, wz, wn = chunks[j]

                def gates(wi, ghidx, bias_ap, func, tag):
                    pg = psums.tile([P, NS], F32, tag="small")
                    for kt in range(kt_in):
                        nc.tensor.matmul(pg[:], wi[:, kt, :], rhs_bf[:, kt, :],
                                         start=(kt == 0), stop=(kt == kt_in - 1))
                    gs = gwork.tile([P, NS], F32, tag=tag + "s")
                    nc.vector.tensor_add(gs[:], pg[:], ghsb[:, ghidx, :])
                    g = gwork.tile([P, NS], F32, tag=tag)
                    nc.scalar.activation(g[:], gs[:], func, bias=bias_ap)
                    return g

                r_t = gates(wr, ghbase + j, brz_sb[:, j, :], AF.Sigmoid, "r")
                z_t = gates(wz, ghbase + MT_H + j, brz_sb[:, MT_H + j, :],
                            AF.Sigmoid, "z")

                pin = psums.tile([P, NS], F32, tag="small")
                for kt in range(kt_in):
                    nc.tensor.matmul(pin[:], wn[:, kt, :], rhs_bf[:, kt, :],
                                     start=(kt == 0), stop=(kt == kt_in - 1))
                hn_t = gwork.tile([P, NS], F32, tag="hn")
                nc.scalar.activation(hn_t[:], ghsb[:, ghbase + 2 * MT_H + j, :],
                                     AF.Identity, bias=bhn_sb[:, j, :])
                t1 = gwork.tile([P, NS], F32, tag="t1")
                nc.vector.tensor_mul(t1[:], r_t[:], hn_t[:])
                t2 = gwork.tile([P, NS], F32, tag="t2")
                nc.vector.tensor_add(t2[:], pin[:], t1[:])
                n_t = gwork.tile([P, NS], F32, tag="nt")
                nc.scalar.activation(n_t[:], t2[:], AF.Tanh,
                                     bias=bin_sb[:, j, :])
                d_t = gwork.tile([P, NS], F32, tag="dt")
                nc.vector.tensor_sub(d_t[:], hf_sb[:, j, :], n_t[:])
                zd = gwork.tile([P, NS], F32, tag="zd")
                nc.vector.tensor_mul(zd[:], z_t[:], d_t[:])
                nc.vector.tensor_add(out_f[:, j, :], n_t[:], zd[:])
                if out_bf is not None:
                    with nc.allow_low_precision(reason="bf16 matmul rhs copy"):
                        nc.vector.tensor_copy(out_bf[:, j, :], out_f[:, j, :])
                # transpose + write this hidden tile now (overlaps next j)
                pt = psums.tile([NS, P], F32, tag="small")
                nc.tensor.transpose(pt[:], out_f[:, j, :], ident[:])
                hid_nat = gwork.tile([NS, P], F32, tag="hidn")
                nc.vector.tensor_copy(hid_nat[:], pt[:])
                nc.sync.dma_start(hid_out[hid_l, :, ds(j * P, P)], hid_nat[:])

        with tc.tile_pool(name="gres", bufs=1) as gres:
            h0p_f = gres.tile([P, KT_H, NS], F32)
            h0p_bf = gres.tile([P, KT_H, NS], BF16)
            h1p_f = gres.tile([P, KT_H, NS], F32)
            gru_layer(l0_chunks, KT_DIN, rnn_bf, h0f_sb, 0,
                      brz0_sb, bin0_sb, bhn0_sb, h0p_f, h0p_bf, 0)
            gru_layer(l1_chunks, KT_H, h0p_bf, h1f_sb, NG,
                      brz1_sb, bin1_sb, bhn1_sb, h1p_f, None, 1)

            # ---- fc (fp32) ----
            pfc = psums.tile([P, NS], F32, tag="small")
            for kt in range(KT_H):
                nc.tensor.matmul(pfc[:], fcw_sb[:, kt, :], h1p_f[:, kt, :],
                                 start=(kt == 0), stop=(kt == KT_H - 1))
            pred_sb = gwork.tile([P, NS], F32, tag="pred")
            nc.scalar.activation(pred_sb[:], pfc[:], AF.Identity,
                                 bias=fcb_sb[:])
            ptr = psums.tile([NS, P], F32, tag="small")
            nc.tensor.transpose(ptr[:], pred_sb[:], ident[:])
            pred_t = gwork.tile([NS, P], F32, tag="predt")
            nc.vector.tensor_copy(pred_t[:], ptr[:])
            nc.sync.dma_start(pred_out[:], pred_t[:])

            nc.sync.dma_start(att_out[:], att_tr[:])

    nc.compile()
    return nc


def _blocked_lhsT(w, n_mt, n_kt, dtype):
    # w: (n_mt*128, n_kt*128) row-major -> [P(k), n_mt, n_kt, P(m)]
    b = w.reshape(n_mt, P, n_kt, P).transpose(3, 0, 2, 1)
    return np.ascontiguousarray(b).astype(dtype)


def _kstack(t, n_kt, dtype):
    # t: (n_kt*128, F) -> [P, n_kt, F]
    b = t.reshape(n_kt, P, -1).transpose(1, 0, 2)
    return np.ascontiguousarray(b).astype(dtype)


def _colvec(v, n_t):
    return np.ascontiguousarray(
        np.asarray(v, np.float32).reshape(n_t, P).T[:, :, None])


BF = ml_dtypes.bfloat16
_prog_cache = {}


def _get_program():
    if "nc" not in _prog_cache:
        _prog_cache["nc"] = build_program()
    return _prog_cache["nc"]


def make_in_maps(x, encoder_states, hidden, emb, U_w, U_b, W_w, W_b,
                 attn_w, W_ih0, W_hh0, b_ih0, b_hh0,
                 W_ih1, W_hh1, b_ih1, b_hh1, fc_w, fc_b):
    f = np.float32
    x = np.asarray(x)
    enc = np.asarray(encoder_states, f)
    hidden = np.asarray(hidden, f)
    shared = {
        "uwT": _blocked_lhsT(np.asarray(U_w, f), MT_H, KT_ENC, BF),
        "wwT": _blocked_lhsT(np.asarray(W_w, f), MT_H, KT_H, BF),
        "ubw": _colvec(np.asarray(U_b, f) + np.asarray(W_b, f), MT_H),
        "attw": _colvec(np.asarray(attn_w, f)[0], MT_H).astype(BF),
        "emb": np.ascontiguousarray(np.asarray(emb, f).reshape(P, 2, P)),
        "wih0T": _blocked_lhsT(np.asarray(W_ih0, f), NG, KT_DIN, BF),
        "whh0T": _blocked_lhsT(np.asarray(W_hh0, f), NG, KT_H, BF),
        "wih1T": _blocked_lhsT(np.asarray(W_ih1, f), NG, KT_H, BF),
        "whh1T": _blocked_lhsT(np.asarray(W_hh1, f), NG, KT_H, BF),
        "brz0": _colvec((np.asarray(b_ih0, f) + np.asarray(b_hh0, f))[:2 * H],
                        2 * MT_H),
        "bin0": _colvec(np.asarray(b_ih0, f)[2 * H:], MT_H),
        "bhn0": _colvec(np.asarray(b_hh0, f)[2 * H:], MT_H),
        "brz1": _colvec((np.asarray(b_ih1, f) + np.asarray(b_hh1, f))[:2 * H],
                        2 * MT_H),
        "bin1": _colvec(np.asarray(b_ih1, f)[2 * H:], MT_H),
        "bhn1": _colvec(np.asarray(b_hh1, f)[2 * H:], MT_H),
        "fcwT": _blocked_lhsT(np.asarray(fc_w, f), 1, KT_H, f)[:, 0],
        "fcb": np.ascontiguousarray(np.asarray(fc_b, f)[:, None]),
    }
    in_maps = []
    for k in range(NC):
        n0 = k * NS
        encs = enc[:, n0:n0 + NS, :]                      # (S, NS, 2H)
        encT2 = encs.transpose(2, 1, 0).reshape(D2, SN)   # [d, (n s)]
        h0 = hidden[0, n0:n0 + NS].T                      # (H, NS)
        h1 = hidden[1, n0:n0 + NS].T
        xs = np.asarray(x[n0:n0 + NS])
        onehot = (xs[None, :] == np.arange(V)[:, None]).astype(f)
        m = dict(shared)
        m.update({
            "encT": _kstack(encT2, KT_ENC, BF),
            "h0T_b": _kstack(h0, KT_H, BF),
            "h1T_b": _kstack(h1, KT_H, BF),
            "h0T_f": _kstack(h0, KT_H, f),
            "h1T_f": _kstack(h1, KT_H, f),
            "onehot": np.ascontiguousarray(onehot),
        })
        in_maps.append(m)
    return in_maps


def assemble_outputs(results):
    pred = np.concatenate([r["pred_out"] for r in results], axis=0)
    hid = np.concatenate([r["hid_out"] for r in results], axis=1)
    att = np.concatenate([r["att_out"] for r in results], axis=1)[:, :, None]
    return (pred.astype(np.float32), hid.astype(np.float32),
            att.astype(np.float32))


def kernel(x, encoder_states, hidden, cell, emb, U_w, U_b, W_w, W_b,
           attn_w, attn_b, W_ih0, W_hh0, b_ih0, b_hh0,
           W_ih1, W_hh1, b_ih1, b_hh1, fc_w, fc_b, **_ignored):
    # attn_b shifts every energy equally -> softmax-invariant; cell unused.
    nc = _get_program()
    in_maps = make_in_maps(x, encoder_states, hidden, emb, U_w, U_b, W_w, W_b,
                           attn_w, W_ih0, W_hh0, b_ih0, b_hh0,
                           W_ih1, W_hh1, b_ih1, b_hh1, fc_w, fc_b)
    res = run_bass_kernel_spmd(nc, in_maps, core_ids=list(range(NC)))
    return assemble_outputs(res.results)


# revision 41
# speedup vs baseline: 1.2007x; 1.0670x over previous
"""Trainium2 Bass kernel for a Bahdanau-attention 2-layer GRU decoder step.

Strategy (data-parallel over batch N across 8 NeuronCores, no collectives):
  - Host pre-arranges inputs: per-core encoder_states shard transposed to
    [d, (n, s)] layout (contraction dim on partitions), weights transposed +
    tile-blocked for the TensorEngine lhsT layout, bf16 for all big matmul
    operands, biases folded.
  - Attention: u = U_w @ enc (dominant matmul) accumulated per 512-wide free
    tile; w broadcast-added on DVE; tanh on ACT (bf16); energy via a bf16 PE
    matmul against attn_w; softmax over s on the free dim; context as a
    mul+reduce over s on DVE against a partition-broadcast attention copy.
  - GRU: the recurrent halves (W_hh @ h) are precomputed during attention,
    filling PE gaps while encoder chunks stream in; results park in SBUF and
    re-seed the gate PSUMs, so the serial GRU phase runs only the W_ih
    matmuls (weights prefetched through a deep SBUF ring during attention).
    Gate math fused on ACT/DVE, outputs transposed on PE per tile.
"""
import numpy as np
import ml_dtypes
from contextlib import ExitStack

import concourse.bass as bass
import concourse.tile as tile
from concourse import bacc, mybir
from concourse.bass_utils import run_bass_kernel_spmd
from concourse.masks import make_identity
from concourse.bass import ds, ts

F32 = mybir.dt.float32
BF16 = mybir.dt.bfloat16
AF = mybir.ActivationFunctionType
OP = mybir.AluOpType
AX = mybir.AxisListType

S, N, H, E, V = 64, 256, 1024, 256, 128
D2, DIN, H3 = 2048, 2304, 3072
NC = 8
NS = N // NC            # 32: per-core batch shard
SN = NS * S             # 2048: flattened (n, s), n-major
P = 128
KT_ENC = D2 // P        # 16
KT_H = H // P           # 8
KT_DIN = DIN // P       # 18
MT_H = H // P           # 8 h m-tiles
FT = 512                # moving free tile for the u matmul
NT = SN // FT           # 4
NB = FT // S            # 8 batch elements per free tile
NG = 3 * MT_H           # 24 gate m-tiles per layer


def build_program():
    nc = bacc.Bacc("TRN2", target_bir_lowering=False, debug=False,
                   num_devices=NC)

    def inp(name, shape, dtype=F32):
        return nc.dram_tensor(name, shape, dtype, kind="ExternalInput").ap()

    def outp(name, shape, dtype=F32):
        return nc.dram_tensor(name, shape, dtype, kind="ExternalOutput").ap()

    encT = inp("encT", [P, KT_ENC, SN], BF16)
    uwT = inp("uwT", [P, MT_H, KT_ENC, P], BF16)
    wwT = inp("wwT", [P, MT_H, KT_H, P], BF16)
    h0T_b = inp("h0T_b", [P, KT_H, NS], BF16)
    h1T_b = inp("h1T_b", [P, KT_H, NS], BF16)
    h0T_f = inp("h0T_f", [P, KT_H, NS])
    h1T_f = inp("h1T_f", [P, KT_H, NS])
    ubw = inp("ubw", [P, MT_H, 1])
    attw = inp("attw", [P, MT_H, 1], BF16)
    onehot = inp("onehot", [P, NS])
    emb = inp("emb", [P, 2, P])
    wih0T = inp("wih0T", [P, NG, KT_DIN, P], BF16)
    whh0T = inp("whh0T", [P, NG, KT_H, P], BF16)
    wih1T = inp("wih1T", [P, NG, KT_H, P], BF16)
    whh1T = inp("whh1T", [P, NG, KT_H, P], BF16)
    brz0 = inp("brz0", [P, 2 * MT_H, 1])
    bin0 = inp("bin0", [P, MT_H, 1])
    bhn0 = inp("bhn0", [P, MT_H, 1])
    brz1 = inp("brz1", [P, 2 * MT_H, 1])
    bin1 = inp("bin1", [P, MT_H, 1])
    bhn1 = inp("bhn1", [P, MT_H, 1])
    fcwT = inp("fcwT", [P, KT_H, P])
    fcb = inp("fcb", [P, 1])

    pred_out = outp("pred_out", [NS, V])
    hid_out = outp("hid_out", [2, NS, H])
    att_out = outp("att_out", [S, NS])

    with tile.TileContext(nc) as tc, ExitStack() as ctx:
        const = ctx.enter_context(tc.tile_pool(name="const", bufs=1))
        psums = ctx.enter_context(tc.tile_pool(name="psums", bufs=3, space="PSUM"))
        gwork = ctx.enter_context(tc.tile_pool(name="gwork", bufs=4))
        gw = ctx.enter_context(tc.tile_pool(name="gw", bufs=12))
        gwhh = ctx.enter_context(tc.tile_pool(name="gwhh", bufs=5))

        def cload(ap_in, shape, dtype=F32):
            t = const.tile(shape, dtype, tag=ap_in.tensor.name)
            nc.sync.dma_start(t[:], ap_in[:])
            return t

        h0b_sb = cload(h0T_b, [P, KT_H, NS], BF16)
        h1b_sb = cload(h1T_b, [P, KT_H, NS], BF16)
        h0f_sb = cload(h0T_f, [P, KT_H, NS])
        h1f_sb = cload(h1T_f, [P, KT_H, NS])
        ubw_sb = cload(ubw, [P, MT_H, 1])
        attw_sb = cload(attw, [P, MT_H, 1], BF16)
        onehot_sb = cload(onehot, [P, NS])
        emb_sb = cload(emb, [P, 2, P])
        brz0_sb = cload(brz0, [P, 2 * MT_H, 1])
        bin0_sb = cload(bin0, [P, MT_H, 1])
        bhn0_sb = cload(bhn0, [P, MT_H, 1])
        brz1_sb = cload(brz1, [P, 2 * MT_H, 1])
        bin1_sb = cload(bin1, [P, MT_H, 1])
        bhn1_sb = cload(bhn1, [P, MT_H, 1])
        fcw_sb = cload(fcwT, [P, KT_H, P])
        fcb_sb = cload(fcb, [P, 1])
        ident = const.tile([P, P], F32)
        make_identity(nc, ident)

        rnn_bf = const.tile([P, KT_DIN, NS], BF16)
        att_tr = const.tile([S, NS], F32)
        # precomputed recurrent halves: L0 r/z/n then L1 r/z/n
        ghsb = const.tile([P, 2 * NG, NS], F32)

        def gh_group(whh_src, g, out_idx, hb):
            c = gwhh.tile([P, KT_H, P], BF16, tag="whh")
            nc.gpsimd.dma_start(c[:], whh_src[:, g, :, :])
            pg = psums.tile([P, NS], F32, tag="small")
            for kt in range(KT_H):
                nc.tensor.matmul(pg[:], c[:, kt, :], hb[:, kt, :],
                                 start=(kt == 0), stop=(kt == KT_H - 1))
            nc.scalar.activation(ghsb[:, out_idx, :], pg[:], AF.Identity)

        gh_jobs = ([(whh0T, g, g, h0b_sb) for g in range(NG)] +
                   [(whh1T, g, NG + g, h1b_sb) for g in range(NG)])

        # ================= attention phase (scoped SBUF) =================
        with tc.tile_pool(name="aenc", bufs=2) as aenc, \
             tc.tile_pool(name="auw", bufs=1) as auw, \
             tc.tile_pool(name="ares", bufs=1) as ares, \
             tc.tile_pool(name="apsum", bufs=5, space="PSUM") as apsum, \
             tc.tile_pool(name="awork", bufs=2) as awork, \
             tc.tile_pool(name="aprod", bufs=1) as aprod, \
             tc.tile_pool(name="attp", bufs=2) as attp, \
             tc.tile_pool(name="adram", bufs=2, space="DRAM") as adram:

            # per-(kt, t) enc chunks and per-m U_w tiles: fine-grained DMA
            # deps so the first u matmuls start as soon as chunks land.
            uw0_sb = auw.tile([P, KT_ENC, P], BF16, tag="uw0", name="uw0")
            nc.sync.dma_start(uw0_sb[:], uwT[:, 0, :, :])
            # enc chunks are double-buffered just-in-time per t: tile t+1
            # loads while t computes, t frees after the context pass reads it.
            enc_sb = [None] * NT

            def load_enc_chunks(t):
                tt = aenc.tile([P, KT_ENC, FT], BF16, tag="enct",
                               name=f"enct{t}")
                nc.sync.dma_start(tt[:], encT[:, :, ds(t * FT, FT)])
                enc_sb[t] = tt

            load_enc_chunks(0)
            uwr_sb = auw.tile([P, MT_H - 1, KT_ENC, P], BF16, tag="uwr",
                              name="uwr")
            nc.sync.dma_start(uwr_sb[:], uwT[:, 1:, :, :])
            uw_sb = [uw0_sb] + [uwr_sb[:, m - 1] for m in range(1, MT_H)]

            # w_T[h, n] = W_w @ h1_shard + (U_b + W_b)
            w_sb = ares.tile([P, MT_H, NS], F32)
            for m in range(MT_H):
                ww_c = awork.tile([P, KT_H, P], BF16, tag="wwc")
                nc.gpsimd.dma_start(ww_c[:], wwT[:, m, :, :])
                pw = psums.tile([P, NS], F32, tag="small")
                for kt in range(KT_H):
                    nc.tensor.matmul(pw[:], ww_c[:, kt, :], h1b_sb[:, kt, :],
                                     start=(kt == 0), stop=(kt == KT_H - 1))
                nc.scalar.activation(w_sb[:, m, :], pw[:], AF.Identity,
                                     bias=ubw_sb[:, m, :])

            # embedding gather via one-hot matmul (exact in fp32)
            for et in range(2):
                pe_ = psums.tile([P, NS], F32, tag="small")
                nc.tensor.matmul(pe_[:], emb_sb[:, et, :], onehot_sb[:],
                                 start=True, stop=True)
                nc.vector.tensor_copy(rnn_bf[:, KT_ENC + et, :], pe_[:])

            # first slice of recurrent-half precompute (fills PE startup)
            for job in gh_jobs[0:12]:
                gh_group(*job)

            # queue the W_ih stream (ring prefetches during attention)
            def wih_chunk(src, g, kt_in):
                t = gw.tile([P, kt_in, P], BF16, tag="wih")
                nc.gpsimd.dma_start(t[:], src[:, g, :, :])
                return t

            l0_chunks = [(wih_chunk(wih0T, j, KT_DIN),
                          wih_chunk(wih0T, MT_H + j, KT_DIN),
                          wih_chunk(wih0T, 2 * MT_H + j, KT_DIN))
                         for j in range(MT_H)]
            l1_chunks = [(wih_chunk(wih1T, j, KT_H),
                          wih_chunk(wih1T, MT_H + j, KT_H),
                          wih_chunk(wih1T, 2 * MT_H + j, KT_H))
                         for j in range(MT_H)]

            for t in range(NT):
                if t + 1 < NT:
                    load_enc_chunks(t + 1)
                pe_en = psums.tile([1, FT], F32, tag="small")
                for m in range(MT_H):
                    pu = apsum.tile([P, FT], F32, tag="pu")
                    for kt in range(KT_ENC):
                        nc.tensor.matmul(pu[:], uw_sb[m][:, kt, :],
                                         enc_sb[t][:, kt, :],
                                         start=(kt == 0),
                                         stop=(kt == KT_ENC - 1))
                    tmp = awork.tile([P, NB, S], BF16, tag="tmp")
                    nc.vector.tensor_tensor(
                        tmp[:], pu[:].rearrange("p (n s) -> p n s", s=S),
                        w_sb[:, m, ds(t * NB, NB), None].to_broadcast(
                            [P, NB, S]),
                        OP.add)
                    th = awork.tile([P, FT], BF16, tag="th")
                    nc.scalar.activation(
                        th[:], tmp[:].rearrange("p n s -> p (n s)"), AF.Tanh)
                    nc.tensor.matmul(pe_en[:], attw_sb[:, m, :], th[:],
                                     start=(m == 0), stop=(m == MT_H - 1),
                                     skip_group_check=True)

                # energy [1, 512] -> [8, 64] (n-part, s-free), softmax over s
                en_sb = attp.tile([1, FT], F32, tag="en")
                nc.vector.tensor_copy(en_sb[:], pe_en[:])
                en_nm = attp.tile([NB, S], F32, tag="ennm")
                nc.sync.dma_start(en_nm[:], en_sb[:])
                mxn = attp.tile([NB, 1], F32, tag="mxn")
                nc.vector.reduce_max(out=mxn[:], in_=en_nm[:], axis=AX.X,
                                     negate=True)
                ex = attp.tile([NB, S], F32, tag="ex")
                se = attp.tile([NB, 1], F32, tag="se")
                nc.scalar.activation(ex[:], en_nm[:], AF.Exp, bias=mxn[:],
                                     accum_out=se[:])
                rc = attp.tile([NB, 1], F32, tag="rc")
                nc.vector.reciprocal(rc[:], se[:])
                att_t = attp.tile([NB, S], F32, tag="attt")
                nc.vector.tensor_scalar_mul(att_t[:], ex[:], rc[:])
                pat = psums.tile([S, NB], F32, tag="small")
                nc.tensor.transpose(pat[:], att_t[:], ident[:NB, :NB])
                nc.vector.tensor_copy(att_tr[:, ds(t * NB, NB)], pat[:])

                # broadcast att across partitions via DRAM bounce (bf16)
                att_tb = attp.tile([NB, S], BF16, tag="attb16")
                with nc.allow_low_precision(reason="attention weights bf16"):
                    nc.vector.tensor_copy(att_tb[:], att_t[:])
                att_d = adram.tile([FT], BF16, tag="attd")
                nc.sync.dma_start(att_d[:], att_tb[:])
                att_b = awork.tile([P, FT], BF16, tag="attb")
                nc.sync.dma_start(att_b[:],
                                  att_d[None, :].to_broadcast([P, FT]))

                # context chunk: rnn[d, n(t)] = sum_s enc[d, n, s] * att[n, s]
                # (runs on the otherwise-idle GpSimd so the DVE queue never
                # blocks PSUM recycling for the u matmuls)
                att_bv = att_b[:].rearrange("p (n s) -> p n s", s=S)
                KH = KT_ENC // 2
                for h2 in range(2):
                    prod = aprod.tile([P, KH, NB, S], BF16, tag="prod")
                    for kk in range(KH):
                        kt = h2 * KH + kk
                        nc.vector.tensor_tensor(
                            prod[:, kk],
                            enc_sb[t][:, kt, :].rearrange(
                                "p (n s) -> p n s", s=S),
                            att_bv, OP.mult)
                    with nc.allow_low_precision(
                            reason="fp32 accumulate, bf16 final write"):
                        nc.vector.reduce_sum(
                            out=rnn_bf[:, ds(h2 * KH, KH), ds(t * NB, NB)],
                            in_=prod[:], axis=AX.X)

                # next slice of recurrent-half precompute (fills PE gaps)
                if t < NT - 1:
                    for job in gh_jobs[12 * (t + 1):12 * (t + 2)]:
                        gh_group(*job)

        # ================= GRU phase (gi matmuls only) ====================
        def gru_layer(chunks, kt_in, rhs_bf, hf_sb, ghbase,
                      brz_sb, bin_sb, bhn_sb, out_f, out_bf, hid_l):
            for j in range(MT_H):
                wr, wz, wn = chunks[j]

                def gates(wi, ghidx, bias_ap, func, tag):
                    pg = psums.tile([P, NS], F32, tag="small")
                    for kt in range(kt_in):
                        nc.tensor.matmul(pg[:], wi[:, kt, :], rhs_bf[:, kt, :],
                                         start=(kt == 0), stop=(kt == kt_in - 1))
                    gs = gwork.tile([P, NS], F32, tag=tag + "s")
                    nc.vector.tensor_add(gs[:], pg[:], ghsb[:, ghidx, :])
                    g = gwork.tile([P, NS], F32, tag=tag)
                    nc.scalar.activation(g[:], gs[:], func, bias=bias_ap)
                    return g

                r_t = gates(wr, ghbase + j, brz_sb[:, j, :], AF.Sigmoid, "r")
                z_t = gates(wz, ghbase + MT_H + j, brz_sb[:, MT_H + j, :],
                            AF.Sigmoid, "z")

                pin = psums.tile([P, NS], F32, tag="small")
                for kt in range(kt_in):
                    nc.tensor.matmul(pin[:], wn[:, kt, :], rhs_bf[:, kt, :],
                                     start=(kt == 0), stop=(kt == kt_in - 1))
                hn_t = gwork.tile([P, NS], F32, tag="hn")
                nc.scalar.activation(hn_t[:], ghsb[:, ghbase + 2 * MT_H + j, :],
                                     AF.Identity, bias=bhn_sb[:, j, :])
                t1 = gwork.tile([P, NS], F32, tag="t1")
                nc.vector.tensor_mul(t1[:], r_t[:], hn_t[:])
                t2 = gwork.tile([P, NS], F32, tag="t2")
                nc.vector.tensor_add(t2[:], pin[:], t1[:])
                n_t = gwork.tile([P, NS], F32, tag="nt")
                nc.scalar.activation(n_t[:], t2[:], AF.Tanh,
                                     bias=bin_sb[:, j, :])
                d_t = gwork.tile([P, NS], F32, tag="dt")
                nc.vector.tensor_sub(d_t[:], hf_sb[:, j, :], n_t[:])
                zd = gwork.tile([P, NS], F32, tag="zd")
                nc.vector.tensor_mul(zd[:], z_t[:], d_t[:])
                nc.vector.tensor_add(out_f[:, j, :], n_t[:], zd[:])
                if out_bf is not None:
                    with nc.allow_low_precision(reason="bf16 matmul rhs copy"):
                        nc.vector.tensor_copy(out_bf[:, j, :], out_f[:, j, :])
                # transpose + write this hidden tile now (overlaps next j)
                pt = psums.tile([NS, P], F32, tag="small")
                nc.tensor.transpose(pt[:], out_f[:, j, :], ident[:])
                hid_nat = gwork.tile([NS, P], F32, tag="hidn")
                nc.vector.tensor_copy(hid_nat[:], pt[:])
                nc.sync.dma_start(hid_out[hid_l, :, ds(j * P, P)], hid_nat[:])

        with tc.tile_pool(name="gres", bufs=1) as gres:
            h0p_f = gres.tile([P, KT_H, NS], F32)
            h0p_bf = gres.tile([P, KT_H, NS], BF16)
            h1p_f = gres.tile([P, KT_H, NS], F32)
            gru_layer(l0_chunks, KT_DIN, rnn_bf, h0f_sb, 0,
                      brz0_sb, bin0_sb, bhn0_sb, h0p_f, h0p_bf, 0)
            gru_layer(l1_chunks, KT_H, h0p_bf, h1f_sb, NG,
                      brz1_sb, bin1_sb, bhn1_sb, h1p_f, None, 1)

            # ---- fc (fp32) ----
            pfc = psums.tile([P, NS], F32, tag="small")
            for kt in range(KT_H):
                nc.tensor.matmul(pfc[:], fcw_sb[:, kt, :], h1p_f[:, kt, :],
                                 start=(kt == 0), stop=(kt == KT_H - 1))
            pred_sb = gwork.tile([P, NS], F32, tag="pred")
            nc.scalar.activation(pred_sb[:], pfc[:], AF.Identity,
                                 bias=fcb_sb[:])
            ptr = psums.tile([NS, P], F32, tag="small")
            nc.tensor.transpose(ptr[:], pred_sb[:], ident[:])
            pred_t = gwork.tile([NS, P], F32, tag="predt")
            nc.vector.tensor_copy(pred_t[:], ptr[:])
            nc.sync.dma_start(pred_out[:], pred_t[:])

            nc.sync.dma_start(att_out[:], att_tr[:])

    nc.compile()
    return nc


def _blocked_lhsT(w, n_mt, n_kt, dtype):
    # w: (n_mt*128, n_kt*128) row-major -> [P(k), n_mt, n_kt, P(m)]
    b = w.reshape(n_mt, P, n_kt, P).transpose(3, 0, 2, 1)
    return np.ascontiguousarray(b).astype(dtype)


def _kstack(t, n_kt, dtype):
    # t: (n_kt*128, F) -> [P, n_kt, F]
    b = t.reshape(n_kt, P, -1).transpose(1, 0, 2)
    return np.ascontiguousarray(b).astype(dtype)


def _colvec(v, n_t):
    return np.ascontiguousarray(
        np.asarray(v, np.float32).reshape(n_t, P).T[:, :, None])


BF = ml_dtypes.bfloat16
_prog_cache = {}


def _get_program():
    if "nc" not in _prog_cache:
        _prog_cache["nc"] = build_program()
    return _prog_cache["nc"]


def make_in_maps(x, encoder_states, hidden, emb, U_w, U_b, W_w, W_b,
                 attn_w, W_ih0, W_hh0, b_ih0, b_hh0,
                 W_ih1, W_hh1, b_ih1, b_hh1, fc_w, fc_b):
    f = np.float32
    x = np.asarray(x)
    enc = np.asarray(encoder_states, f)
    hidden = np.asarray(hidden, f)
    shared = {
        "uwT": _blocked_lhsT(np.asarray(U_w, f), MT_H, KT_ENC, BF),
        "wwT": _blocked_lhsT(np.asarray(W_w, f), MT_H, KT_H, BF),
        "ubw": _colvec(np.asarray(U_b, f) + np.asarray(W_b, f), MT_H),
        "attw": _colvec(np.asarray(attn_w, f)[0], MT_H).astype(BF),
        "emb": np.ascontiguousarray(np.asarray(emb, f).reshape(P, 2, P)),
        "wih0T": _blocked_lhsT(np.asarray(W_ih0, f), NG, KT_DIN, BF),
        "whh0T": _blocked_lhsT(np.asarray(W_hh0, f), NG, KT_H, BF),
        "wih1T": _blocked_lhsT(np.asarray(W_ih1, f), NG, KT_H, BF),
        "whh1T": _blocked_lhsT(np.asarray(W_hh1, f), NG, KT_H, BF),
        "brz0": _colvec((np.asarray(b_ih0, f) + np.asarray(b_hh0, f))[:2 * H],
                        2 * MT_H),
        "bin0": _colvec(np.asarray(b_ih0, f)[2 * H:], MT_H),
        "bhn0": _colvec(np.asarray(b_hh0, f)[2 * H:], MT_H),
        "brz1": _colvec((np.asarray(b_ih1, f) + np.asarray(b_hh1, f))[:2 * H],
                        2 * MT_H),
        "bin1": _colvec(np.asarray(b_ih1, f)[2 * H:], MT_H),
        "bhn1": _colvec(np.asarray(b_hh1, f)[2 * H:], MT_H),
        "fcwT": _blocked_lhsT(np.asarray(fc_w, f), 1, KT_H, f)[:, 0],
        "fcb": np.ascontiguousarray(np.asarray(fc_b, f)[:, None]),
    }
    in_maps = []
    for k in range(NC):
        n0 = k * NS
        encs = enc[:, n0:n0 + NS, :]                      # (S, NS, 2H)
        encT2 = encs.transpose(2, 1, 0).reshape(D2, SN)   # [d, (n s)]
        h0 = hidden[0, n0:n0 + NS].T                      # (H, NS)
        h1 = hidden[1, n0:n0 + NS].T
        xs = np.asarray(x[n0:n0 + NS])
        onehot = (xs[None, :] == np.arange(V)[:, None]).astype(f)
        m = dict(shared)
        m.update({
            "encT": _kstack(encT2, KT_ENC, BF),
            "h0T_b": _kstack(h0, KT_H, BF),
            "h1T_b": _kstack(h1, KT_H, BF),
            "h0T_f": _kstack(h0, KT_H, f),
            "h1T_f": _kstack(h1, KT_H, f),
            "onehot": np.ascontiguousarray(onehot),
        })
        in_maps.append(m)
    return in_maps


def assemble_outputs(results):
    pred = np.concatenate([r["pred_out"] for r in results], axis=0)
    hid = np.concatenate([r["hid_out"] for r in results], axis=1)
    att = np.concatenate([r["att_out"] for r in results], axis=1)[:, :, None]
    return (pred.astype(np.float32), hid.astype(np.float32),
            att.astype(np.float32))


def kernel(x, encoder_states, hidden, cell, emb, U_w, U_b, W_w, W_b,
           attn_w, attn_b, W_ih0, W_hh0, b_ih0, b_hh0,
           W_ih1, W_hh1, b_ih1, b_hh1, fc_w, fc_b, **_ignored):
    # attn_b shifts every energy equally -> softmax-invariant; cell unused.
    nc = _get_program()
    in_maps = make_in_maps(x, encoder_states, hidden, emb, U_w, U_b, W_w, W_b,
                           attn_w, W_ih0, W_hh0, b_ih0, b_hh0,
                           W_ih1, W_hh1, b_ih1, b_hh1, fc_w, fc_b)
    res = run_bass_kernel_spmd(nc, in_maps, core_ids=list(range(NC)))
    return assemble_outputs(res.results)
